# revision 47
# baseline (speedup 1.0000x reference)
"""Block-diagonal linear y = x @ W_blockdiag.T + bias on 8 TRN2 NeuronCores.

Expert-parallel sharding: core k owns diagonal block k — x[:, 512k:512(k+1)],
weight_blocks[k] (512x512), bias[512k:512(k+1)] — and produces the matching
output column slice y[:, 512k:512(k+1)]. No collectives.

Per-core kernel (Tile framework):
  - load x in staggered chunks; within a chunk partition p holds g
    consecutive DRAM rows ("(p g) c"), so every DMA descriptor is a fully
    contiguous stripe (max DMA efficiency)
  - PE-transpose each [128,128] sub-block of an x tile into PSUM (float32r,
    1.5 cyc/row), evacuate as a [128, 512] strip to SBUF (rounding cast,
    alternating DVE/ACT) -> xT blocks [c=128, n=128]
  - 4 accumulating matmuls per token tile: stationary lhsT = xT block,
    moving rhs = W.T strip [c=128, r=512], float32r (1 cyc/row)
  - bias add fused into the PSUM->SBUF evacuation on DVE
  - x loads on the SP HWDGE ring, y stores on GpSimd SWDGE (own sequencer,
    no head-of-line blocking), casts on DVE/ACT
  - identity arrives as a host-supplied input (no GpSimd setup chain);
    a PE warm-up burst of dummy transposes flips the HAM clock gate to
    8/8 before the real matmuls start
"""

import os
import sys

import numpy as np

for _p in ("/opt/trn_rl_repo", "/root/.axon_site/_ro/trn_rl_repo"):
    if os.path.isdir(_p) and _p not in sys.path:
        sys.path.insert(0, _p)

import concourse.bass as bass
import concourse.mybir as mybir
import concourse.tile as tile
from concourse.bass_utils import run_bass_kernel_spmd
from concourse.masks import make_identity
from concourse.tile_rust import add_dep_helper

# Problem shape (hardcoded per spec nn_BlockDiagLinear_19490561590005)
N = 8192          # tokens
D = 4096          # model dim
NB = 8            # diagonal blocks == number of cores
B = 512           # block size (rows == cols)
P = 128           # SBUF partitions
CB = B // P       # 4 contraction chunks of 128
NT = N // P       # 64 token tiles

F32 = mybir.dt.float32
# float32r: 1 cycle/row on the PE for free dim >= 256 (vs 4 for float32)
MM_DT = getattr(mybir.dt, os.environ.get("BD_MM_DT", "float32r"))

# token tiles per DMA chunk (see "(p g) c" note above: x-load and y-store
# chunk boundaries must coincide). Small first chunks = fast pipeline fill;
# small last chunks = short tail.
SCHED = [2, 2, 2, 2] + [4] * 12 + [2, 2, 2, 1, 1]
assert sum(SCHED) == NT
PRELOAD_CHUNKS = 4
WARMUP_TRANSPOSES = 24  # ~3us of PE busy -> HAM at 8/8 when real work lands

_CACHE = {}


def _build_bass():
    nc = bass.Bass("TRN2", target_bir_lowering=False)
    x_d = nc.dram_tensor("x", [N, B], MM_DT, kind="ExternalInput")
    w_d = nc.dram_tensor("w", [B, B], MM_DT, kind="ExternalInput")
    b_d = nc.dram_tensor("b", [B], F32, kind="ExternalInput")
    y_d = nc.dram_tensor("y", [N, B], F32, kind="ExternalOutput")

    with tile.TileContext(nc) as tc:
        with (
            tc.tile_pool(name="const", bufs=1) as const_pool,
            tc.tile_pool(name="xin", bufs=6) as x_pool,
            tc.tile_pool(name="yout", bufs=5) as y_pool,
            tc.tile_pool(name="xT", bufs=4) as xT_pool,
            tc.tile_pool(name="psT", bufs=4, space="PSUM") as psT_pool,
            tc.tile_pool(name="psY", bufs=3, space="PSUM") as psY_pool,
            tc.tile_pool(name="psDummy", bufs=1, space="PSUM") as psD_pool,
        ):
            chunk_of = {}
            acc = 0
            for g in SCHED:
                chunk_of[acc] = g
                acc += g

            def load_x_chunk(t, g):
                x_big = x_pool.tile([P, g * B], MM_DT, tag="xbig")
                nc.sync.dma_start(
                    out=x_big.rearrange("p (g c) -> p g c", g=g),
                    in_=x_d.ap()[t * P : (t + g) * P, :].rearrange(
                        "(p g) c -> p g c", g=g
                    ),
                )
                return x_big

            # DMA issue order on the SP HWDGE ring is FIFO: W row-blocks
            # first (the longest dependency chain: load -> 16 transposes ->
            # 4 copies -> first matmul), then the first x chunks; bias last
            # (only needed by the first ADD).
            w_nat = const_pool.tile([P, CB * B], MM_DT)
            preloaded = {}
            with tc.high_priority():
                for rj in range(CB):
                    nc.sync.dma_start(
                        out=w_nat[:, rj * B : (rj + 1) * B],
                        in_=w_d.ap()[rj * P : (rj + 1) * P, :],
                    )
                for t in sorted(chunk_of)[:PRELOAD_CHUNKS]:
                    preloaded[t] = load_x_chunk(t, chunk_of[t])

            # identity built on GpSimd (no DMA dependency), rounded copy on
            # DVE for the fp32r transposes
            ident_f32 = const_pool.tile([P, P], F32)
            make_identity(nc, ident_f32)
            ident = const_pool.tile([P, P], MM_DT)
            nc.vector.tensor_copy(out=ident, in_=ident_f32)

            bias_rep = const_pool.tile([P, B], F32)
            nc.sync.dma_start(
                out=bias_rep,
                in_=b_d.ap().unsqueeze(0).partition_broadcast(P),
            )

            # PE warm-up burst: dummy transposes reading only the identity.
            # Runs while the W/x DMAs are still in flight and flips the HAM
            # clock gate to 8/8; also absorbs the identity DMA wait so later
            # PE instructions carry at most one fresh semaphore wait each.
            ps_dummy = psD_pool.tile([P, P], MM_DT)
            dummy_inst = nc.tensor.transpose(ps_dummy, ident, ident)
            for _ in range(WARMUP_TRANSPOSES - 1):
                nc.tensor.transpose(ps_dummy, ident, ident)

            def transpose_tile(x_big, base, t):
                xs = x_big[:, (t - base) * B : (t - base + 1) * B]
                psx = psT_pool.tile([P, B], MM_DT, tag="ps_t")
                for ci in range(CB):
                    t_inst = nc.tensor.transpose(
                        psx[:, ci * P : (ci + 1) * P],
                        xs[:, ci * P : (ci + 1) * P],
                        ident,
                    )
                    if t == 0 and ci == 0:
                        add_dep_helper(
                            t_inst.ins, dummy_inst.ins, sync=False,
                            reason="warmup before first x transpose",
                        )
                xT = xT_pool.tile([P, B], MM_DT, tag="xT")
                # alternate the rounding cast between DVE and ACT to keep
                # the DVE under the DMA roofline
                if t % 2 == 0:
                    nc.vector.tensor_copy(out=xT, in_=psx)
                else:
                    nc.scalar.copy(out=xT, in_=psx)
                return xT

            prework = {}

            # wT strips: wT[:, ci*512 + r] (c on partitions) = W[r, ci*128+c]
            wT = const_pool.tile([P, CB * B], MM_DT)
            for ci in range(CB):
                psT = psT_pool.tile([P, B], MM_DT, tag="ps_t")
                for rj in range(CB):
                    nc.tensor.transpose(
                        psT[:, rj * P : (rj + 1) * P],
                        w_nat[:, rj * B + ci * P : rj * B + ci * P + P],
                        ident,
                    )
                nc.scalar.copy(out=wT[:, ci * B : (ci + 1) * B], in_=psT)

            # main loop over 64 token tiles, chunked per SCHED
            x_big = None
            y_big = None
            base = 0
            for t in range(NT):
                if t in chunk_of:
                    g = chunk_of[t]
                    base = t
                    x_big = preloaded.pop(t, None)
                    if x_big is None:
                        x_big = load_x_chunk(t, g)
                    y_big = y_pool.tile([P, g * B], F32, tag="ybig")

                xT = prework.pop(t, None)
                if xT is None:
                    xT = transpose_tile(x_big, base, t)

                psy = psY_pool.tile([P, B], F32)
                for ci in range(CB):
                    nc.tensor.matmul(
                        psy,
                        xT[:, ci * P : (ci + 1) * P],
                        wT[:, ci * B : (ci + 1) * B],
                        start=(ci == 0),
                        stop=(ci == CB - 1),
                    )
                # fused bias add + PSUM->SBUF evacuation
                nc.vector.tensor_add(
                    y_big[:, (t - base) * B : (t - base + 1) * B],
                    psy,
                    bias_rep,
                )

                if t - base == chunk_of[base] - 1:
                    g = chunk_of[base]
                    # y stores go out on the ACT HWDGE ring so they never
                    # block x loads in the SP ring's FIFO
                    nc.scalar.dma_start(
                        out=y_d.ap()[base * P : (base + g) * P, :].rearrange(
                            "(p g) c -> p g c", g=g
                        ),
                        in_=y_big.rearrange("p (g c) -> p g c", g=g),
                    )

    return nc


def _split_pe_multiwaits(nc):
    """Hoist extra sync waits off engine instructions onto sequencer NoOps.

    This walrus build supports only a single attached sync wait per
    instruction; codegen fails with "Too many sync wait commands" otherwise.
    A wait-carrying NoOp immediately before the instruction on the same
    sequencer is semantically identical (the sequencer executes in order).
    """
    k = 0
    for f in nc.m.functions:
        for blk in f.blocks:
            out = []
            changed = False
            for inst in blk.instructions:
                si = inst.sync_info
                if si is not None and len(si.on_wait) > 1:
                    waits = list(si.on_wait)
                    for w in waits[:-1]:
                        nop = mybir.InstNoOp(
                            name=f"I-waitsplit-{k}", ins=[], outs=[]
                        )
                        k += 1
                        nop.engine = inst.engine
                        nop.sync_info = mybir.SyncInfo(on_wait=[w], on_update=[])
                        out.append(nop)
                    inst.sync_info = mybir.SyncInfo(
                        on_wait=[waits[-1]], on_update=list(si.on_update)
                    )
                    changed = True
                out.append(inst)
            if changed:
                blk.instructions = out
    return nc


def _get_nc():
    if "nc" not in _CACHE:
        _CACHE["nc"] = _split_pe_multiwaits(_build_bass())
    return _CACHE["nc"]


_IDENT = None


def _run(inputs, trace=False):
    global _IDENT
    x = np.ascontiguousarray(np.asarray(inputs["x"], dtype=np.float32))
    w = np.ascontiguousarray(np.asarray(inputs["weight_blocks"], dtype=np.float32))
    bias = np.ascontiguousarray(np.asarray(inputs["bias"], dtype=np.float32))
    assert x.shape == (N, D) and w.shape == (NB, B, B) and bias.shape == (D,)
    nc = _get_nc()
    in_maps = [
        {
            "x": np.ascontiguousarray(x[:, k * B : (k + 1) * B]),
            "w": np.ascontiguousarray(w[k]),
            "b": np.ascontiguousarray(bias[k * B : (k + 1) * B]),
        }
        for k in range(NB)
    ]
    try:
        res = run_bass_kernel_spmd(
            nc, in_maps, core_ids=list(range(NB)), trace=trace
        )
    except Exception:
        # the axon-tunneled devices occasionally report a transient
        # NRT_EXEC_UNIT_UNRECOVERABLE; a single retry has always recovered
        res = run_bass_kernel_spmd(
            nc, in_maps, core_ids=list(range(NB)), trace=trace
        )
    y = np.concatenate([res.results[k]["y"] for k in range(NB)], axis=1)
    return np.asarray(y, dtype=np.float32), res


def kernel(**inputs):
    y, _ = _run(inputs, trace=False)
    return y


def kernel_traced(**inputs):
    return _run(inputs, trace=True)
